# revision 9
# baseline (speedup 1.0000x reference)
"""Trainium2 Bass kernel for nn_CustomConv2D (degenerate conv: only the last
input channel contributes; 3x3 VALID conv -> 64 out channels + bias).

Strategy (v10 — fp8 DoubleRow matmuls, fp8 in/out, balanced V/S evictions):
  - The problem is HBM-traffic bound and the tolerance is 2e-2. The bias
    (~N(0,1)) dominates the output magnitude while the conv part has RMS
    ~0.3, so the kernel stores the BIAS-FREE conv result as fp8-e4m3
    (6.42 MB/core) and the host adds the bias in f32. Input and weights
    are fp8 too (sim rel err ~1.3e-2 vs the 2e-2 gate).
  - Matmuls run in fp8 DoubleRow mode (0.5 cycles/output-column): the
    K=18 contraction pairs each tap k of image A with tap k of image B.
    DoubleRow wants the pair as a 2-entry BLOCKED free dim (middle-dim
    byte step %16==0; an interleaved stride-1 layout hard-crashes the
    core): weights [9p, (t c)] -> [9,2,128], moving [9p, (t n)] ->
    [9,2,N] with t-blocks SEGW apart. Validated bit-exact on HW.
  - Quadrant s of each pair tile holds im2col rows on partitions
    32s..32s+8 (tile_position rows must be 32-aligned). Input is compact
    in HBM (0.90 MB/core): one [9 x 6272] 56 KB DMA per (pair, seg).
  - PSUM->SBUF(fp8) evictions are the throughput wall: only DVE
    (~0.96 GHz/lane from PSUM) and ACT (~1.2 GHz/lane) can read PSUM.
    Matmul pairs write [128,1024] two-bank PSUM tiles at 512-col pitch;
    one strided-AP eviction covers both banks. The V/S assignment
    alternates per segment so each engine gets 7 tiles per two segments.
  - Scalar and Vector issue NO DMAs. Sync HWDGE carries w2 + pair 0/1
    loads + pair 0/1 drains; GpSimd SWDGE (otherwise idle) carries pair
    2/3 loads + drains. The final seg drains as two halves on both
    queues to shorten the tail.
"""

import sys

if "/opt/trn_rl_repo" not in sys.path:
    sys.path.insert(0, "/opt/trn_rl_repo")

import numpy as np
import ml_dtypes

B, CIN, COUT, KS = 64, 64, 64, 3
H, W, HP, WP = 112, 112, 114, 114
NPIX = H * W          # 12544
NCORES = 8
BL = B // NCORES      # 8 local batches per core
PAIRS = BL // 2       # 4
KT = KS * KS          # 9 taps; DoubleRow pairs imgA/imgB per tap
NSEG = 4              # pixel segments per pair (partition offsets 0/32/64/96)
SEGW = NPIX // NSEG   # 3136
NT = 448              # pixels per matmul; 7 * 448 == 3136, fits one PSUM bank
TPS = SEGW // NT      # 7 matmul tiles per segment

_CACHE = {}


def _build_bass():
    import concourse.bass as bass
    import concourse.bacc as bacc
    import concourse.mybir as mybir
    from concourse.tile import TileContext

    f32 = mybir.dt.float32
    f8 = mybir.dt.float8e4
    # Bacc (not plain Bass): its compile() runs move_matmul_waits_to_ldweights
    # + generate_event_semaphores, without which walrus rejects any sync wait
    # on a Matmult ("Too many sync wait commands").
    nc = bacc.Bacc("TRN2", target_bir_lowering=False, debug=False)
    mv = nc.declare_dram_parameter("mv", [PAIRS, NSEG, KT, 2 * SEGW], f8,
                                   isOutput=False)
    w2 = nc.declare_dram_parameter("w2", [128, 256], f8, isOutput=False)
    out = nc.declare_dram_parameter("out", [BL * COUT, NPIX], f8,
                                    isOutput=True)

    with TileContext(nc) as tc:
        with (
            tc.tile_pool(name="consts", bufs=1) as consts,
            tc.tile_pool(name="movp", bufs=PAIRS) as movp,
            tc.tile_pool(name="stagep", bufs=4 * PAIRS) as stagep,
            # 3x two-bank tiles + 2x one-bank tiles = 8 PSUM banks exactly.
            tc.tile_pool(name="psum2", bufs=3, space="PSUM") as psum2,
            tc.tile_pool(name="psum1", bufs=2, space="PSUM") as psum1,
        ):
            w2_t = consts.tile([128, 256], f8)
            movs = [movp.tile([128, 2 * SEGW], f8, tag="mov",
                              name=f"mov{p}") for p in range(PAIRS)]

            # Weights + pair 0 seg 0 first (they gate the first matmul).
            # Sync: w2, pairs 0-1; GpSimd: pairs 2-3.
            nc.sync.dma_start(out=w2_t[:], in_=w2[:])
            for p in range(PAIRS):
                eng = nc.sync if p < 2 else nc.gpsimd
                for s in range(NSEG):
                    eng.dma_start(out=movs[p][32 * s:32 * s + KT, :],
                                  in_=mv[p, s])

            def mm(ps_tile, col0, pair, seg, t):
                p0 = 32 * seg
                n0 = t * NT
                wap = w2_t[p0:p0 + KT, :].rearrange("p (t c) -> p t c", t=2)
                map_ = movs[pair][p0:p0 + KT, :].rearrange(
                    "p (t n) -> p t n", t=2)[:, :, n0:n0 + NT]
                nc.tensor.matmul(ps_tile[:, col0:col0 + NT], wap, map_,
                                 start=True, stop=True,
                                 perf_mode=mybir.MatmulPerfMode.DoubleRow,
                                 tile_position=(p0, 0))

            def evict2(eng, ps_tile, stage, t0):
                # Two-bank strided PSUM read -> contiguous fp8 stage cols.
                src = ps_tile[:, :].rearrange("p (g c) -> p g c", c=512)
                src = src[:, :, 0:NT]
                dst = stage[:, t0 * NT:(t0 + 2) * NT].rearrange(
                    "p (g c) -> p g c", c=NT)
                if eng == "v":
                    nc.vector.tensor_scalar_add(dst, src, 0.0)
                else:
                    nc.scalar.copy(dst, src)

            def evict1(eng, ps_tile, stage):
                dst = stage[:, 6 * NT:SEGW]
                if eng == "v":
                    nc.vector.tensor_scalar_add(dst, ps_tile[:, 0:NT], 0.0)
                else:
                    nc.scalar.copy(dst, ps_tile[:, 0:NT])

            for pair in range(PAIRS):
                stages = [stagep.tile([128, SEGW], f8, tag="stage",
                                      name=f"stage_{pair}_{s}")
                          for s in range(NSEG)]
                for seg in range(NSEG):
                    st = stages[seg]
                    # Alternate which engine gets the heavier 2+2 share.
                    first_v = (pair * NSEG + seg) % 2 == 0
                    eA, eB, eC, eD = (("v", "s", "v", "s") if first_v
                                      else ("s", "v", "s", "v"))
                    psA = psum2.tile([128, 1024], f32, tag="ps2")
                    mm(psA, 0, pair, seg, 0)
                    mm(psA, 512, pair, seg, 1)
                    evict2(eA, psA, st, 0)
                    psB = psum2.tile([128, 1024], f32, tag="ps2")
                    mm(psB, 0, pair, seg, 2)
                    mm(psB, 512, pair, seg, 3)
                    evict2(eB, psB, st, 2)
                    psC = psum2.tile([128, 1024], f32, tag="ps2")
                    mm(psC, 0, pair, seg, 4)
                    mm(psC, 512, pair, seg, 5)
                    evict2(eC, psC, st, 4)
                    psD = psum1.tile([128, 512], f32, tag="psD")
                    mm(psD, 0, pair, seg, 6)
                    evict1(eD, psD, st)
                    # Per-seg 401 KB drains: Sync takes pairs 0,1; GpSimd
                    # takes pairs 2,3 (matching their load queues). The
                    # very last seg drains as two halves on both queues.
                    orow = pair * 128
                    ocol = seg * SEGW
                    last = (pair == PAIRS - 1 and seg == NSEG - 1)
                    if last:
                        half = SEGW // 2
                        nc.sync.dma_start(
                            out=out[orow:orow + 128, ocol:ocol + half],
                            in_=st[:, 0:half])
                        nc.gpsimd.dma_start(
                            out=out[orow:orow + 128,
                                    ocol + half:ocol + SEGW],
                            in_=st[:, half:SEGW])
                    else:
                        eng = nc.sync if pair < 2 else nc.gpsimd
                        eng.dma_start(
                            out=out[orow:orow + 128, ocol:ocol + SEGW],
                            in_=st[:, :])
    nc.compile()
    return nc


def _get_nc():
    if "nc" not in _CACHE:
        _CACHE["nc"] = _build_bass()
    return _CACHE["nc"]


def _prep_inputs(x_padded, weight, bias):
    x = np.asarray(x_padded, dtype=np.float32)
    wt = np.asarray(weight, dtype=np.float32)

    xs3 = x[:, -1, :, :]                              # [64, 114, 114]
    win = np.lib.stride_tricks.sliding_window_view(xs3, (KS, KS), axis=(1, 2))
    # [64, 112, 112, 3, 3] -> [64, 9, 12544] with row k = (i, j) shift
    mov_all = win.transpose(0, 3, 4, 1, 2).reshape(B, KT, NPIX)
    # [c, pair, t(img2), ki, seg, n] -> [c, pair, seg, ki, t, n]
    mov_r = mov_all.reshape(NCORES, PAIRS, 2, KT, NSEG, SEGW)
    mov_h = np.ascontiguousarray(
        mov_r.transpose(0, 1, 4, 3, 2, 5)
    ).reshape(NCORES, PAIRS, NSEG, KT, 2 * SEGW).astype(
        ml_dtypes.float8_e4m3)

    # DoubleRow weights: quadrant s partitions 32s+ki hold the t=0 block
    # [wl | 0] in cols 0:128 and the t=1 block [0 | wl] in cols 128:256.
    wl = np.ascontiguousarray(wt[:, -1, :, :]).reshape(COUT, KT)
    w2 = np.zeros((128, 256), np.float32)
    for s in range(NSEG):
        w2[32 * s: 32 * s + KT, 0:64] = wl.T
        w2[32 * s: 32 * s + KT, 192:256] = wl.T
    w2 = w2.astype(ml_dtypes.float8_e4m3)
    return mov_h, w2


def kernel(x_padded, weight, bias, in_height=112, in_width=112, **_unused):
    from concourse.bass_utils import run_bass_kernel_spmd

    mov_h, w2 = _prep_inputs(x_padded, weight, bias)
    nc = _get_nc()
    in_maps = [
        {"mv": mov_h[c], "w2": w2}
        for c in range(NCORES)
    ]
    res = run_bass_kernel_spmd(nc, in_maps, core_ids=list(range(NCORES)))
    bs = np.asarray(bias, dtype=np.float32)
    outs = [
        np.asarray(res.results[c]["out"]).astype(np.float32)
        .reshape(BL, COUT, H, W)
        for c in range(NCORES)
    ]
    full = np.concatenate(outs, axis=0)              # conv only, no bias
    return full + bs[None, :, None, None]


# revision 11
# speedup vs baseline: 1.0060x; 1.0060x over previous
"""Trainium2 Bass kernel for nn_CustomConv2D (degenerate conv: only the last
input channel contributes; 3x3 VALID conv -> 64 out channels + bias).

Strategy (v11 — fp8 DoubleRow, quadrant-interleaved, balanced V/S evictions):
  - The problem is HBM-traffic bound and the tolerance is 2e-2. The bias
    (~N(0,1)) dominates the output magnitude while the conv part has RMS
    ~0.3, so the kernel stores the BIAS-FREE conv result as fp8-e4m3
    (6.42 MB/core) and the host adds the bias in f32. Input and weights
    are fp8 too (measured rel err 1.44e-2 vs the 2e-2 gate).
  - Matmuls run in fp8 DoubleRow mode (2 contraction rows/cell): the
    K=18 contraction pairs each tap k of image A with tap k of image B
    as a 2-entry BLOCKED free dim (middle-dim byte step %16==0; an
    interleaved stride-1 layout hard-crashes the core). Validated
    bit-exact on HW. MM cost ~187 ns per 448-col tile at the 1.2 GHz
    mid p-state.
  - DoubleRow turns FWL off, so each LDWEIGHTS costs ~197 ns and does
    NOT hide behind a matmul on the SAME PE quadrant (v10 measured
    384 ns/tile). Fix: segments are processed in PAIRS with their
    matmuls interleaved A0,B0,A1,B1,... — every LDWEIGHTS targets the
    opposite 32-row quadrant of the running matmul, so the 64-deep
    reorder window pulls it ahead and hides it.
  - PSUM->SBUF(fp8) evictions are the throughput wall: only DVE
    (~0.96 GHz/lane from PSUM) and ACT (~1.2 GHz/lane) can read PSUM.
    Matmul pairs write [128,1024] two-bank PSUM tiles at 512-col pitch;
    one strided-AP eviction covers both banks. Per seg-pair each engine
    gets 7 tiles (3 two-bank groups + 1 single).
  - Scalar and Vector issue NO DMAs. Sync HWDGE carries w2 + pair 0/2
    loads + pair 0/2 drains; GpSimd SWDGE carries pair 1/3. Input rows
    are padded 9->16 per quadrant ([16 x 6272] 100 KB loads span 4 SDMA
    ports; 1.6 MB/core). The final seg drains as two halves on both
    queues to shorten the tail.
"""

import sys

if "/opt/trn_rl_repo" not in sys.path:
    sys.path.insert(0, "/opt/trn_rl_repo")

import numpy as np
import ml_dtypes

B, CIN, COUT, KS = 64, 64, 64, 3
H, W, HP, WP = 112, 112, 114, 114
NPIX = H * W          # 12544
NCORES = 8
BL = B // NCORES      # 8 local batches per core
PAIRS = BL // 2       # 4
KT = KS * KS          # 9 taps; DoubleRow pairs imgA/imgB per tap
KR = 16               # quadrant rows loaded (9 data + 7 zero pad)
NSEG = 4              # pixel segments per pair (partition offsets 0/32/64/96)
SEGW = NPIX // NSEG   # 3136
NT = 448              # pixels per matmul; 7 * 448 == 3136, fits one PSUM bank
TPS = SEGW // NT      # 7 matmul tiles per segment

_CACHE = {}


def _build_bass():
    import concourse.bass as bass
    import concourse.bacc as bacc
    import concourse.mybir as mybir
    from concourse.tile import TileContext

    f32 = mybir.dt.float32
    f8 = mybir.dt.float8e4
    # Bacc (not plain Bass): its compile() runs move_matmul_waits_to_ldweights
    # + generate_event_semaphores, without which walrus rejects any sync wait
    # on a Matmult ("Too many sync wait commands").
    nc = bacc.Bacc("TRN2", target_bir_lowering=False, debug=False)
    mv = nc.declare_dram_parameter("mv", [PAIRS, NSEG, KR, 2 * SEGW], f8,
                                   isOutput=False)
    w2 = nc.declare_dram_parameter("w2", [128, 256], f8, isOutput=False)
    out = nc.declare_dram_parameter("out", [BL * COUT, NPIX], f8,
                                    isOutput=True)

    with TileContext(nc) as tc:
        with (
            tc.tile_pool(name="consts", bufs=1) as consts,
            tc.tile_pool(name="movp", bufs=PAIRS) as movp,
            tc.tile_pool(name="stagep", bufs=4 * PAIRS) as stagep,
            # Two seg streams (A/B), each 2x two-bank tiles = 8 PSUM
            # banks exactly; the 7th (448-col) tile of each seg cycles
            # through the same pools.
            tc.tile_pool(name="psumA", bufs=2, space="PSUM") as psumA,
            tc.tile_pool(name="psumB", bufs=2, space="PSUM") as psumB,
        ):
            w2_t = consts.tile([128, 256], f8)
            movs = [movp.tile([128, 2 * SEGW], f8, tag="mov",
                              name=f"mov{p}") for p in range(PAIRS)]

            # Weights + pair 0 seg 0/1 first (they gate the first matmuls).
            # Sync: w2, pairs 0,2; GpSimd: pairs 1,3.
            nc.sync.dma_start(out=w2_t[:], in_=w2[:])
            for p in range(PAIRS):
                eng = nc.sync if p % 2 == 0 else nc.gpsimd
                for s in range(NSEG):
                    eng.dma_start(out=movs[p][32 * s:32 * s + KR, :],
                                  in_=mv[p, s])

            def mm(ps_tile, col0, pair, seg, t):
                p0 = 32 * seg
                n0 = t * NT
                wap = w2_t[p0:p0 + KT, :].rearrange("p (t c) -> p t c", t=2)
                map_ = movs[pair][p0:p0 + KT, :].rearrange(
                    "p (t n) -> p t n", t=2)[:, :, n0:n0 + NT]
                nc.tensor.matmul(ps_tile[:, col0:col0 + NT], wap, map_,
                                 start=True, stop=True,
                                 perf_mode=mybir.MatmulPerfMode.DoubleRow,
                                 tile_position=(p0, 0))

            def evict2(eng, ps_tile, stage, t0):
                # Two-bank strided PSUM read -> contiguous fp8 stage cols.
                src = ps_tile[:, :].rearrange("p (g c) -> p g c", c=512)
                src = src[:, :, 0:NT]
                dst = stage[:, t0 * NT:(t0 + 2) * NT].rearrange(
                    "p (g c) -> p g c", c=NT)
                if eng == "v":
                    nc.vector.tensor_scalar_add(dst, src, 0.0)
                else:
                    nc.scalar.copy(dst, src)

            def evict1(eng, ps_tile, stage):
                dst = stage[:, 6 * NT:SEGW]
                if eng == "v":
                    nc.vector.tensor_scalar_add(dst, ps_tile[:, 0:NT], 0.0)
                else:
                    nc.scalar.copy(dst, ps_tile[:, 0:NT])

            def drain(st, pair, seg):
                orow = pair * 128
                ocol = seg * SEGW
                last = (pair == PAIRS - 1 and seg == NSEG - 1)
                if last:
                    half = SEGW // 2
                    nc.sync.dma_start(
                        out=out[orow:orow + 128, ocol:ocol + half],
                        in_=st[:, 0:half])
                    nc.gpsimd.dma_start(
                        out=out[orow:orow + 128, ocol + half:ocol + SEGW],
                        in_=st[:, half:SEGW])
                else:
                    eng = nc.sync if pair % 2 == 0 else nc.gpsimd
                    eng.dma_start(
                        out=out[orow:orow + 128, ocol:ocol + SEGW],
                        in_=st[:, :])

            for pair in range(PAIRS):
                stages = [stagep.tile([128, SEGW], f8, tag="stage",
                                      name=f"stage_{pair}_{s}")
                          for s in range(NSEG)]
                for sp in range(NSEG // 2):       # seg-pairs (0,1), (2,3)
                    sA, sB = 2 * sp, 2 * sp + 1
                    stA, stB = stages[sA], stages[sB]
                    # Interleave A/B matmuls so every LDWEIGHTS targets
                    # the idle quadrant. Evictions: V and S each get 7
                    # tiles per seg-pair.
                    fv = (pair + sp) % 2 == 0
                    e = ("v", "s") if fv else ("s", "v")
                    psa = psumA.tile([128, 1024], f32, tag="psA")
                    psb = psumB.tile([128, 1024], f32, tag="psB")
                    mm(psa, 0, pair, sA, 0)
                    mm(psb, 0, pair, sB, 0)
                    mm(psa, 512, pair, sA, 1)
                    mm(psb, 512, pair, sB, 1)
                    evict2(e[0], psa, stA, 0)
                    evict2(e[1], psb, stB, 0)
                    psa2 = psumA.tile([128, 1024], f32, tag="psA")
                    psb2 = psumB.tile([128, 1024], f32, tag="psB")
                    mm(psa2, 0, pair, sA, 2)
                    mm(psb2, 0, pair, sB, 2)
                    mm(psa2, 512, pair, sA, 3)
                    mm(psb2, 512, pair, sB, 3)
                    evict2(e[1], psa2, stA, 2)
                    evict2(e[0], psb2, stB, 2)
                    psa3 = psumA.tile([128, 1024], f32, tag="psA")
                    psb3 = psumB.tile([128, 1024], f32, tag="psB")
                    mm(psa3, 0, pair, sA, 4)
                    mm(psb3, 0, pair, sB, 4)
                    mm(psa3, 512, pair, sA, 5)
                    mm(psb3, 512, pair, sB, 5)
                    evict2(e[0], psa3, stA, 4)
                    evict2(e[1], psb3, stB, 4)
                    psd1 = psumA.tile([128, 1024], f32, tag="psA")
                    psd2 = psumB.tile([128, 1024], f32, tag="psB")
                    mm(psd1, 0, pair, sA, 6)
                    mm(psd2, 0, pair, sB, 6)
                    evict1(e[1], psd1, stA)
                    evict1(e[0], psd2, stB)
                    drain(stA, pair, sA)
                    drain(stB, pair, sB)
    nc.compile()
    return nc


def _get_nc():
    if "nc" not in _CACHE:
        _CACHE["nc"] = _build_bass()
    return _CACHE["nc"]


def _prep_inputs(x_padded, weight, bias):
    x = np.asarray(x_padded, dtype=np.float32)
    wt = np.asarray(weight, dtype=np.float32)

    xs3 = x[:, -1, :, :]                              # [64, 114, 114]
    win = np.lib.stride_tricks.sliding_window_view(xs3, (KS, KS), axis=(1, 2))
    # [64, 112, 112, 3, 3] -> [64, 9, 12544] with row k = (i, j) shift
    mov_all = win.transpose(0, 3, 4, 1, 2).reshape(B, KT, NPIX)
    # [c, pair, t(img2), ki, seg, n] -> [c, pair, seg, ki, t, n]
    mov_r = mov_all.reshape(NCORES, PAIRS, 2, KT, NSEG, SEGW)
    mov_k = np.ascontiguousarray(
        mov_r.transpose(0, 1, 4, 3, 2, 5)
    ).reshape(NCORES, PAIRS, NSEG, KT, 2 * SEGW)
    mov_h = np.zeros((NCORES, PAIRS, NSEG, KR, 2 * SEGW), np.float32)
    mov_h[:, :, :, :KT, :] = mov_k
    mov_h = mov_h.astype(ml_dtypes.float8_e4m3)

    # DoubleRow weights: quadrant s partitions 32s+ki hold the t=0 block
    # [wl | 0] in cols 0:128 and the t=1 block [0 | wl] in cols 128:256.
    wl = np.ascontiguousarray(wt[:, -1, :, :]).reshape(COUT, KT)
    w2 = np.zeros((128, 256), np.float32)
    for s in range(NSEG):
        w2[32 * s: 32 * s + KT, 0:64] = wl.T
        w2[32 * s: 32 * s + KT, 192:256] = wl.T
    w2 = w2.astype(ml_dtypes.float8_e4m3)
    return mov_h, w2


def kernel(x_padded, weight, bias, in_height=112, in_width=112, **_unused):
    from concourse.bass_utils import run_bass_kernel_spmd

    mov_h, w2 = _prep_inputs(x_padded, weight, bias)
    nc = _get_nc()
    in_maps = [
        {"mv": mov_h[c], "w2": w2}
        for c in range(NCORES)
    ]
    res = run_bass_kernel_spmd(nc, in_maps, core_ids=list(range(NCORES)))
    bs = np.asarray(bias, dtype=np.float32)
    outs = [
        np.asarray(res.results[c]["out"]).astype(np.float32)
        .reshape(BL, COUT, H, W)
        for c in range(NCORES)
    ]
    full = np.concatenate(outs, axis=0)              # conv only, no bias
    return full + bs[None, :, None, None]


# revision 12
# speedup vs baseline: 1.0869x; 1.0804x over previous
"""Trainium2 Bass kernel for nn_CustomConv2D (degenerate conv: only the last
input channel contributes; 3x3 VALID conv -> 64 out channels + bias).

Strategy (v12 — fp8 in/out, balanced V/S evictions, lean DMA/semaphores):
  - The problem is HBM-traffic bound and the tolerance is 2e-2. The bias
    (~N(0,1)) dominates the output magnitude while the conv part has RMS
    ~0.3, so the kernel stores the BIAS-FREE conv result as fp8-e4m3
    (6.42 MB/core) and the host adds the bias in f32. The im2col input is
    fp8 (1.61 MB/core incl. quadrant padding; a packed partition-split AP
    load mis-places data at runtime, so the zero-padded [128 x 3136]
    whole-tile load per pair stays). Measured end-to-end rel err ~1.2e-2.
  - Each matmul is [18 -> 128, 448] at PE quadrant offsets 0/32/64/96
    (tile_position rows must be 32-aligned). PSUM output is hard-capped
    at one 2KB bank per matmul (ISA), so N=448. f16 stationary keeps FWL
    on so LDWEIGHTS hides behind the matmuls (fp8 DoubleRow halves the
    MM cycles but disables FWL; its exposed 197 ns LDWEIGHTS and the
    extra PSUM double-buffering stalls made it a net loss, v10/v11).
  - PSUM->SBUF(fp8) evictions are the throughput wall: only DVE
    (0.96 GHz/lane from PSUM) and ACT (1.2 GHz/lane) can read PSUM, one
    elem/cycle/lane each. Matmul pairs write a [128,1024] two-bank PSUM
    tile at 512-col pitch; one strided-AP eviction covers both banks
    (measured: V 1086 ns, S 1030 ns per 896-col group). The V/S
    assignment alternates per segment so each engine gets 7 tiles per
    two segments (~86% busy both at the PE-mid-p-state pace).
  - Scalar and Vector issue NO DMAs (they must never see ring stalls).
    Input loads + half the drains ride the Sync HWDGE ring; the other
    drains ride GpSimd SWDGE (otherwise idle). Pair 0's seg-0 rows load
    first, then the weights (both gate the first matmul), then the rest;
    drains are per-seg 401 KB, and the final seg drains as two halves
    both on Sync (GpSimd dispatch lags ~1us at the tail).
"""

import sys

if "/opt/trn_rl_repo" not in sys.path:
    sys.path.insert(0, "/opt/trn_rl_repo")

import numpy as np
import ml_dtypes

B, CIN, COUT, KS = 64, 64, 64, 3
H, W, HP, WP = 112, 112, 114, 114
NPIX = H * W          # 12544
NCORES = 8
BL = B // NCORES      # 8 local batches per core
PAIRS = BL // 2       # 4
KDIM = 2 * KS * KS    # 18 (9 taps x 2 images, block-diagonal weights)
NSEG = 4              # pixel segments per pair (partition offsets 0/32/64/96)
SEGW = NPIX // NSEG   # 3136
NT = 448              # pixels per matmul; 7 * 448 == 3136, fits one PSUM bank
TPS = SEGW // NT      # 7 matmul tiles per segment

_CACHE = {}


def _build_bass():
    import concourse.bass as bass
    import concourse.bacc as bacc
    import concourse.mybir as mybir
    from concourse.tile import TileContext

    f32 = mybir.dt.float32
    f16 = mybir.dt.float16
    f8 = mybir.dt.float8e4
    # Bacc (not plain Bass): its compile() runs move_matmul_waits_to_ldweights
    # + generate_event_semaphores, without which walrus rejects any sync wait
    # on a Matmult ("Too many sync wait commands").
    nc = bacc.Bacc("TRN2", target_bir_lowering=False, debug=False)
    mv = nc.declare_dram_parameter("mv", [PAIRS, 128, SEGW], f8,
                                   isOutput=False)
    w2 = nc.declare_dram_parameter("w2", [128, 128], f16, isOutput=False)
    out = nc.declare_dram_parameter("out", [BL * COUT, NPIX], f8,
                                    isOutput=True)

    with TileContext(nc) as tc:
        with (
            tc.tile_pool(name="consts", bufs=1) as consts,
            tc.tile_pool(name="movp", bufs=PAIRS) as movp,
            tc.tile_pool(name="stagep", bufs=4 * PAIRS) as stagep,
            # 3x two-bank tiles + 2x one-bank tiles = 8 PSUM banks exactly.
            tc.tile_pool(name="psum2", bufs=3, space="PSUM") as psum2,
            tc.tile_pool(name="psum1", bufs=2, space="PSUM") as psum1,
        ):
            w2_t = consts.tile([128, 128], f16)
            movs = [movp.tile([128, SEGW], f8, tag="mov",
                              name=f"mov{p}") for p in range(PAIRS)]

            # Pair 0's seg-0 rows land first as a small fast DMA, then the
            # weights (both gate the first matmul), then everything else.
            nc.sync.dma_start(out=movs[0][0:32, :], in_=mv[0, 0:32])
            nc.sync.dma_start(out=w2_t[:], in_=w2[:])
            nc.sync.dma_start(out=movs[0][32:128, :], in_=mv[0, 32:128])
            for p in range(1, PAIRS):
                nc.sync.dma_start(out=movs[p][:, :], in_=mv[p])

            def mm(ps_tile, col0, pair, seg, t):
                p0 = 32 * seg
                n0 = t * NT
                nc.tensor.matmul(ps_tile[:, col0:col0 + NT],
                                 w2_t[p0:p0 + KDIM, :],
                                 movs[pair][p0:p0 + KDIM, n0:n0 + NT],
                                 start=True, stop=True,
                                 tile_position=(p0, 0))

            def evict2(eng, ps_tile, stage, t0):
                # Two-bank strided PSUM read -> contiguous fp8 stage cols.
                src = ps_tile[:, :].rearrange("p (g c) -> p g c", c=512)
                src = src[:, :, 0:NT]
                dst = stage[:, t0 * NT:(t0 + 2) * NT].rearrange(
                    "p (g c) -> p g c", c=NT)
                if eng == "v":
                    nc.vector.tensor_scalar_add(dst, src, 0.0)
                else:
                    nc.scalar.copy(dst, src)

            def evict1(eng, ps_tile, stage):
                dst = stage[:, 6 * NT:SEGW]
                if eng == "v":
                    nc.vector.tensor_scalar_add(dst, ps_tile[:, 0:NT], 0.0)
                else:
                    nc.scalar.copy(dst, ps_tile[:, 0:NT])

            for pair in range(PAIRS):
                stages = [stagep.tile([128, SEGW], f8, tag="stage",
                                      name=f"stage_{pair}_{s}")
                          for s in range(NSEG)]
                for seg in range(NSEG):
                    st = stages[seg]
                    # Alternate which engine gets the heavier 2+2 share.
                    first_v = (pair * NSEG + seg) % 2 == 0
                    eA, eB, eC, eD = (("v", "s", "v", "s") if first_v
                                      else ("s", "v", "s", "v"))
                    psA = psum2.tile([128, 1024], f32, tag="ps2")
                    mm(psA, 0, pair, seg, 0)
                    mm(psA, 512, pair, seg, 1)
                    evict2(eA, psA, st, 0)
                    psB = psum2.tile([128, 1024], f32, tag="ps2")
                    mm(psB, 0, pair, seg, 2)
                    mm(psB, 512, pair, seg, 3)
                    evict2(eB, psB, st, 2)
                    psC = psum2.tile([128, 1024], f32, tag="ps2")
                    mm(psC, 0, pair, seg, 4)
                    mm(psC, 512, pair, seg, 5)
                    evict2(eC, psC, st, 4)
                    psD = psum1.tile([128, 512], f32, tag="psD")
                    mm(psD, 0, pair, seg, 6)
                    evict1(eD, psD, st)
                    # Per-seg 401 KB drains: Sync takes pairs 0,2; GpSimd
                    # (otherwise idle) takes pairs 1,3. The very last seg
                    # drains as two halves, both on Sync.
                    orow = pair * 128
                    ocol = seg * SEGW
                    last = (pair == PAIRS - 1 and seg == NSEG - 1)
                    if last:
                        half = SEGW // 2
                        nc.sync.dma_start(
                            out=out[orow:orow + 128, ocol:ocol + half],
                            in_=st[:, 0:half])
                        nc.sync.dma_start(
                            out=out[orow:orow + 128,
                                    ocol + half:ocol + SEGW],
                            in_=st[:, half:SEGW])
                    else:
                        eng = nc.sync if pair % 2 == 0 else nc.gpsimd
                        eng.dma_start(
                            out=out[orow:orow + 128, ocol:ocol + SEGW],
                            in_=st[:, :])
    nc.compile()
    return nc


def _get_nc():
    if "nc" not in _CACHE:
        _CACHE["nc"] = _build_bass()
    return _CACHE["nc"]


def _prep_inputs(x_padded, weight, bias):
    x = np.asarray(x_padded, dtype=np.float32)
    wt = np.asarray(weight, dtype=np.float32)

    xs3 = x[:, -1, :, :]                              # [64, 114, 114]
    win = np.lib.stride_tricks.sliding_window_view(xs3, (KS, KS), axis=(1, 2))
    # [64, 112, 112, 3, 3] -> [64, 9, 12544] with row k = (i, j) shift
    mov_all = win.transpose(0, 3, 4, 1, 2).reshape(B, KS * KS, NPIX)
    # [cores, pairs, img2, 9, seg, SEGW] -> [cores, pairs, seg, (img2, 9), SEGW]
    mov_r = mov_all.reshape(NCORES, PAIRS, 2, KS * KS, NSEG, SEGW)
    mov_k = mov_r.transpose(0, 1, 4, 2, 3, 5).reshape(
        NCORES, PAIRS, NSEG, KDIM, SEGW)
    # Pad each 18-row seg block to the 32-row PE quadrant: [.., 4, 32, SEGW]
    mov_h = np.zeros((NCORES, PAIRS, NSEG, 32, SEGW), np.float32)
    mov_h[:, :, :, :KDIM, :] = mov_k
    mov_h = mov_h.reshape(NCORES, PAIRS, 128, SEGW).astype(
        ml_dtypes.float8_e4m3)

    wl = np.ascontiguousarray(wt[:, -1, :, :]).reshape(COUT, KS * KS)
    w2 = np.zeros((128, 128), np.float32)
    for s in range(NSEG):
        w2[32 * s: 32 * s + 9, 0:64] = wl.T
        w2[32 * s + 9: 32 * s + 18, 64:128] = wl.T
    w2 = w2.astype(np.float16)
    return mov_h, w2


def kernel(x_padded, weight, bias, in_height=112, in_width=112, **_unused):
    from concourse.bass_utils import run_bass_kernel_spmd

    mov_h, w2 = _prep_inputs(x_padded, weight, bias)
    nc = _get_nc()
    in_maps = [
        {"mv": mov_h[c], "w2": w2}
        for c in range(NCORES)
    ]
    res = run_bass_kernel_spmd(nc, in_maps, core_ids=list(range(NCORES)))
    bs = np.asarray(bias, dtype=np.float32)
    outs = [
        np.asarray(res.results[c]["out"]).astype(np.float32)
        .reshape(BL, COUT, H, W)
        for c in range(NCORES)
    ]
    full = np.concatenate(outs, axis=0)              # conv only, no bias
    return full + bs[None, :, None, None]


# revision 14
# speedup vs baseline: 1.1259x; 1.0359x over previous
"""Trainium2 Bass kernel for nn_CustomConv2D (degenerate conv: only the last
input channel contributes; 3x3 VALID conv -> 64 out channels + bias).

Strategy (v13 — fp8 in/out, balanced V/S evictions, lean DMA/semaphores):
  - The problem is HBM-traffic bound and the tolerance is 2e-2. The bias
    (~N(0,1)) dominates the output magnitude while the conv part has RMS
    ~0.3, so the kernel stores the BIAS-FREE conv result as fp8-e4m3
    (6.42 MB/core) and the host adds the bias in f32. The im2col input is
    fp8 (1.61 MB/core incl. quadrant padding; a packed partition-split AP
    load mis-places data at runtime, so the zero-padded [128 x 3136]
    whole-tile load per pair stays). Measured end-to-end rel err ~1.2e-2.
  - Each matmul is [18 -> 128, 448] at PE quadrant offsets 0/32/64/96
    (tile_position rows must be 32-aligned). PSUM output is hard-capped
    at one 2KB bank per matmul (ISA), so N=448. f16 stationary keeps FWL
    on so LDWEIGHTS hides behind the matmuls (fp8 DoubleRow halves the
    MM cycles but disables FWL; its exposed 197 ns LDWEIGHTS and the
    extra PSUM double-buffering stalls made it a net loss, v10/v11).
  - PSUM->SBUF(fp8) evictions are the throughput wall: only DVE
    (0.96 GHz/lane from PSUM) and ACT (1.2 GHz/lane) can read PSUM, one
    elem/cycle/lane each. Matmul pairs write a [128,1024] two-bank PSUM
    tile at 512-col pitch; one strided-AP eviction covers both banks
    (measured: V 1086 ns, S 1030 ns per 896-col group). The V/S
    assignment alternates per segment so each engine gets 7 tiles per
    two segments (~86% busy both at the PE-mid-p-state pace).
  - Scalar and Vector issue NO DMAs (they must never see ring stalls).
    Input loads + half the drains ride the Sync HWDGE ring; the other
    drains ride GpSimd SWDGE (otherwise idle). Pair 0's seg-0 rows load
    first, then the weights (both gate the first matmul), then the rest;
    drains are per-seg 401 KB, and the final seg drains as two halves
    both on Sync (GpSimd dispatch lags ~1us at the tail).
"""

import sys

if "/opt/trn_rl_repo" not in sys.path:
    sys.path.insert(0, "/opt/trn_rl_repo")

import numpy as np
import ml_dtypes

B, CIN, COUT, KS = 64, 64, 64, 3
H, W, HP, WP = 112, 112, 114, 114
NPIX = H * W          # 12544
NCORES = 8
BL = B // NCORES      # 8 local batches per core
PAIRS = BL // 2       # 4
KDIM = 2 * KS * KS    # 18 (9 taps x 2 images, block-diagonal weights)
NSEG = 4              # pixel segments per pair (partition offsets 0/32/64/96)
SEGW = NPIX // NSEG   # 3136
NT = 448              # pixels per matmul; 7 * 448 == 3136, fits one PSUM bank
TPS = SEGW // NT      # 7 matmul tiles per segment

_CACHE = {}


def _build_bass():
    import concourse.bass as bass
    import concourse.bacc as bacc
    import concourse.mybir as mybir
    from concourse.tile import TileContext

    f32 = mybir.dt.float32
    f16 = mybir.dt.float16
    f8 = mybir.dt.float8e4
    # Bacc (not plain Bass): its compile() runs move_matmul_waits_to_ldweights
    # + generate_event_semaphores, without which walrus rejects any sync wait
    # on a Matmult ("Too many sync wait commands").
    nc = bacc.Bacc("TRN2", target_bir_lowering=False, debug=False)
    mv = nc.declare_dram_parameter("mv", [PAIRS, 128, SEGW], f8,
                                   isOutput=False)
    # w2 padded to 512 cols: a [128,128] f16 load is 256 B/partition,
    # below the 512 B SDMA line-rate minimum (measured ~2.4us for 32 KB).
    w2 = nc.declare_dram_parameter("w2", [128, 512], f16, isOutput=False)
    out = nc.declare_dram_parameter("out", [BL * COUT, NPIX], f8,
                                    isOutput=True)

    with TileContext(nc) as tc:
        with (
            tc.tile_pool(name="consts", bufs=1) as consts,
            tc.tile_pool(name="movp", bufs=PAIRS) as movp,
            tc.tile_pool(name="stagep", bufs=4 * PAIRS) as stagep,
            # 3x two-bank tiles + 2x one-bank tiles = 8 PSUM banks exactly.
            tc.tile_pool(name="psum2", bufs=3, space="PSUM") as psum2,
            tc.tile_pool(name="psum1", bufs=2, space="PSUM") as psum1,
        ):
            w2_t = consts.tile([128, 512], f16)
            movs = [movp.tile([128, SEGW], f8, tag="mov",
                              name=f"mov{p}") for p in range(PAIRS)]

            # Pair 0's seg-0 rows land first as a small fast DMA, then the
            # weights (both gate the first matmul), then everything else.
            nc.sync.dma_start(out=movs[0][0:32, :], in_=mv[0, 0:32])
            nc.sync.dma_start(out=w2_t[:], in_=w2[:])
            nc.sync.dma_start(out=movs[0][32:128, :], in_=mv[0, 32:128])
            for p in range(1, PAIRS):
                nc.sync.dma_start(out=movs[p][:, :], in_=mv[p])

            def mm(ps_tile, col0, pair, seg, t):
                p0 = 32 * seg
                n0 = t * NT
                nc.tensor.matmul(ps_tile[:, col0:col0 + NT],
                                 w2_t[p0:p0 + KDIM, 0:128],
                                 movs[pair][p0:p0 + KDIM, n0:n0 + NT],
                                 start=True, stop=True,
                                 tile_position=(p0, 0))

            def evict2(eng, ps_tile, stage, t0):
                # Two-bank strided PSUM read -> contiguous fp8 stage cols.
                src = ps_tile[:, :].rearrange("p (g c) -> p g c", c=512)
                src = src[:, :, 0:NT]
                dst = stage[:, t0 * NT:(t0 + 2) * NT].rearrange(
                    "p (g c) -> p g c", c=NT)
                if eng == "v":
                    nc.vector.tensor_scalar_add(dst, src, 0.0)
                else:
                    nc.scalar.copy(dst, src)

            def evict1(eng, ps_tile, stage):
                dst = stage[:, 6 * NT:SEGW]
                if eng == "v":
                    nc.vector.tensor_scalar_add(dst, ps_tile[:, 0:NT], 0.0)
                else:
                    nc.scalar.copy(dst, ps_tile[:, 0:NT])

            for pair in range(PAIRS):
                stages = [stagep.tile([128, SEGW], f8, tag="stage",
                                      name=f"stage_{pair}_{s}")
                          for s in range(NSEG)]
                for seg in range(NSEG):
                    st = stages[seg]
                    # Alternate which engine gets the heavier 2+2 share.
                    first_v = (pair * NSEG + seg) % 2 == 0
                    eA, eB, eC, eD = (("v", "s", "v", "s") if first_v
                                      else ("s", "v", "s", "v"))
                    psA = psum2.tile([128, 1024], f32, tag="ps2")
                    mm(psA, 0, pair, seg, 0)
                    mm(psA, 512, pair, seg, 1)
                    evict2(eA, psA, st, 0)
                    psB = psum2.tile([128, 1024], f32, tag="ps2")
                    mm(psB, 0, pair, seg, 2)
                    mm(psB, 512, pair, seg, 3)
                    evict2(eB, psB, st, 2)
                    psC = psum2.tile([128, 1024], f32, tag="ps2")
                    mm(psC, 0, pair, seg, 4)
                    mm(psC, 512, pair, seg, 5)
                    evict2(eC, psC, st, 4)
                    psD = psum1.tile([128, 512], f32, tag="psD")
                    mm(psD, 0, pair, seg, 6)
                    evict1(eD, psD, st)
                    # Per-seg 401 KB drains: Sync takes pairs 0,2; GpSimd
                    # (otherwise idle) takes pairs 1,3. The very last seg
                    # drains as two halves, both on Sync.
                    orow = pair * 128
                    ocol = seg * SEGW
                    last = (pair == PAIRS - 1 and seg == NSEG - 1)
                    if last:
                        half = SEGW // 2
                        nc.sync.dma_start(
                            out=out[orow:orow + 128, ocol:ocol + half],
                            in_=st[:, 0:half])
                        nc.sync.dma_start(
                            out=out[orow:orow + 128,
                                    ocol + half:ocol + SEGW],
                            in_=st[:, half:SEGW])
                    else:
                        eng = nc.sync if pair % 2 == 0 else nc.gpsimd
                        eng.dma_start(
                            out=out[orow:orow + 128, ocol:ocol + SEGW],
                            in_=st[:, :])
    nc.compile()
    return nc


def _get_nc():
    if "nc" not in _CACHE:
        _CACHE["nc"] = _build_bass()
    return _CACHE["nc"]


def _prep_inputs(x_padded, weight, bias):
    x = np.asarray(x_padded, dtype=np.float32)
    wt = np.asarray(weight, dtype=np.float32)

    xs3 = x[:, -1, :, :]                              # [64, 114, 114]
    win = np.lib.stride_tricks.sliding_window_view(xs3, (KS, KS), axis=(1, 2))
    # [64, 112, 112, 3, 3] -> [64, 9, 12544] with row k = (i, j) shift
    mov_all = win.transpose(0, 3, 4, 1, 2).reshape(B, KS * KS, NPIX)
    # [cores, pairs, img2, 9, seg, SEGW] -> [cores, pairs, seg, (img2, 9), SEGW]
    mov_r = mov_all.reshape(NCORES, PAIRS, 2, KS * KS, NSEG, SEGW)
    mov_k = mov_r.transpose(0, 1, 4, 2, 3, 5).reshape(
        NCORES, PAIRS, NSEG, KDIM, SEGW)
    # Pad each 18-row seg block to the 32-row PE quadrant: [.., 4, 32, SEGW]
    mov_h = np.zeros((NCORES, PAIRS, NSEG, 32, SEGW), np.float32)
    mov_h[:, :, :, :KDIM, :] = mov_k
    mov_h = mov_h.reshape(NCORES, PAIRS, 128, SEGW).astype(
        ml_dtypes.float8_e4m3)

    wl = np.ascontiguousarray(wt[:, -1, :, :]).reshape(COUT, KS * KS)
    w2 = np.zeros((128, 512), np.float32)
    for s in range(NSEG):
        w2[32 * s: 32 * s + 9, 0:64] = wl.T
        w2[32 * s + 9: 32 * s + 18, 64:128] = wl.T
    w2 = w2.astype(np.float16)
    return mov_h, w2


def kernel(x_padded, weight, bias, in_height=112, in_width=112, **_unused):
    from concourse.bass_utils import run_bass_kernel_spmd

    mov_h, w2 = _prep_inputs(x_padded, weight, bias)
    nc = _get_nc()
    in_maps = [
        {"mv": mov_h[c], "w2": w2}
        for c in range(NCORES)
    ]
    res = run_bass_kernel_spmd(nc, in_maps, core_ids=list(range(NCORES)))
    bs = np.asarray(bias, dtype=np.float32)
    outs = [
        np.asarray(res.results[c]["out"]).astype(np.float32)
        .reshape(BL, COUT, H, W)
        for c in range(NCORES)
    ]
    full = np.concatenate(outs, axis=0)              # conv only, no bias
    return full + bs[None, :, None, None]
